# revision 36
# baseline (speedup 1.0000x reference)
"""Trainium2 Bass kernel for nn_CpxRNN: 64-step RNN over B=4096 samples,
data-parallel across 8 NeuronCores (512 samples/core).

Math (per core, b = sample, columns of every on-chip tile):
  state kept transposed+shifted: nh = (elu(z)+1)^T  as [128, 1024] bf16
  (hidden chunk m in cols [512m, 512m+512), hidden unit i = 128*m + p).
  elu(z)+1 == min(exp(z), max(z+1, 1))  -- 1 ACT + 2 DVE ops, bias folded
  into the ACT/TS immediates.  All "-1" corrections from the shift are
  folded into host-precomputed biases (b~ = b - colsum(W)).
  One-hot input term reduced to rank-1: prevoh @ W_in = W_in[0] + x*delta.
  Heads (logits 2 + phase 4 rows) are one M=6 matmul pair per step written
  at 32-aligned partition offsets, 4 steps per PSUM bank, DMA-flushed to
  SBUF.  Post-processing does log-softmax + phase head with pattern
  matmuls (pair-sum / diff / even-select / block-diag W_ph2).
"""

import sys

sys.path.insert(0, "/opt/trn_rl_repo")

from contextlib import ExitStack

import ml_dtypes
import numpy as np

import concourse.bass as bass
import concourse.tile as tile
from concourse import bacc, mybir
from concourse.bass_utils import run_bass_kernel_spmd

L = 64
H = 256
B = 4096
NCORES = 8
BL = B // NCORES  # 512
F32 = mybir.dt.float32
F32R = mybir.dt.float32r
BF16 = mybir.dt.bfloat16
AF = mybir.ActivationFunctionType
OP = mybir.AluOpType
BF = ml_dtypes.bfloat16

# ---------------------------------------------------------------- host side


def _host_constants(W_in, W_carry, b_carry, W_prob, b_prob, W_ph1, b_ph1,
                    W_ph2, b_ph2):
    c = {}
    W_in = W_in.astype(np.float64)
    W_carry = W_carry.astype(np.float64)
    # state bias: b_carry - colsum(W_carry) + W_in[0]
    bias_state_vec = b_carry - W_carry.sum(0) + W_in[0]

    # initial shifted state (t=0 uses zero prev input, zero h), half layout
    nh0_vec = np.where(b_carry > 0, b_carry, np.expm1(b_carry)) + 1.0  # [256]
    nh0 = np.empty((128, 512), np.float32)
    for m in range(2):
        nh0[:, 256 * m:256 * m + 256] = nh0_vec[128 * m:128 * m + 128][:, None]
    c["nh0"] = nh0.astype(BF)

    c["wc"] = W_carry.astype(np.float32).astype(BF)          # [256, 256] lhsT
    # K=3 augmented input matmul: lhsT rows = [delta; bias_hi; bias_lo]
    # (bias split so the bf16 lhsT carries it at ~f32 accuracy)
    bp1 = bias_state_vec + 1.0   # PSUM carries z+1 for the fused elu tail
    b_hi = bp1.astype(np.float32).astype(BF).astype(np.float64)
    b_lo = bp1 - b_hi
    db = np.stack([W_in[1] - W_in[0], b_hi, b_lo])            # [3, 256]
    c["db"] = db.astype(np.float32).astype(BF)

    W_head = np.concatenate([W_prob, W_ph1], axis=1)          # [256, 6]
    b_head = np.concatenate([b_prob, b_ph1])                  # [6]
    c["wh"] = W_head.astype(np.float32).astype(BF)
    whp = np.zeros((256, 128), np.float32)
    whp[:, :6] = W_head
    c["whp"] = whp.astype(BF)
    bh6 = (b_head - W_head.astype(np.float64).sum(0))         # [6]
    bh96 = np.tile(bh6, 16).reshape(96, 1)
    c["bias_head"] = bh96.astype(np.float32)
    c["bias_head_p1"] = (bh96 + 1.0).astype(np.float32)

    # phase2 bias: b_ph2 - colsum(W_ph2)
    by4 = b_ph2 - W_ph2.astype(np.float64).sum(0)             # [4]
    by = np.tile(by4, 32).reshape(128, 1)
    c["bias_y"] = by.astype(np.float32)
    c["bias_y_p1"] = (by + 1.0).astype(np.float32)

    # pattern lhsTs over head tiles.  Head tile tau holds steps
    # t = 16*tau + tt at partitions 6*tt + r (r: 0,1 logits; 2..5 phase).
    pattS = np.zeros((96, 256), np.float32)
    pattD = np.zeros((96, 256), np.float32)
    pattL0 = np.zeros((96, 256), np.float32)
    pattW2 = np.zeros((96, 512), np.float32)
    for tau in range(4):
        for tt in range(16):
            t = 16 * tau + tt
            pattS[6 * tt + 0, 64 * tau + t] = 1.0
            pattS[6 * tt + 1, 64 * tau + t] = 1.0
            pattD[6 * tt + 0, 64 * tau + t] = -1.0
            pattD[6 * tt + 1, 64 * tau + t] = 1.0
            pattL0[6 * tt + 0, 64 * tau + t] = 1.0
            h = tau // 2
            for i in range(4):
                for j in range(4):
                    q = 4 * (t - 32 * h) + j   # out partition within half h
                    pattW2[6 * tt + 2 + i, 128 * tau + q] = W_ph2[i, j]
    c["pattS"] = pattS.astype(BF)
    c["pattD"] = pattD.astype(BF)
    c["pattL0"] = pattL0.astype(BF)
    c["pattW2"] = pattW2.astype(BF)
    c["ones128"] = np.ones((128, 1), np.float32).astype(BF)
    c["neg1"] = np.full((128, 1), -1.0, np.float32)
    return c


_IN_SPECS = [
    # name, shape, dtype
    ("x_aug", (3, L * BL), BF16),
    ("x_f32", (L, BL), F32),
    ("nh0", (128, 512), BF16),
    ("wc", (256, 256), BF16),
    ("db", (3, 256), BF16),
    ("wh", (256, 6), BF16),
    ("whp", (256, 128), BF16),
    ("bias_head", (96, 1), F32),
    ("bias_head_p1", (96, 1), F32),
    ("bias_y", (128, 1), F32),
    ("bias_y_p1", (128, 1), F32),
    ("pattS", (96, 256), BF16),
    ("pattD", (96, 256), BF16),
    ("pattL0", (96, 256), BF16),
    ("pattW2", (96, 512), BF16),
    ("ones128", (128, 1), BF16),
    ("neg1", (128, 1), F32),
]

# ---------------------------------------------------------------- device side


def _build_kernel(ctx: ExitStack, tc: tile.TileContext, io: dict):
    nc = tc.nc
    sb = ctx.enter_context(tc.tile_pool(name="sb", bufs=1))
    st = ctx.enter_context(tc.tile_pool(name="st", bufs=2))
    tmp = ctx.enter_context(tc.tile_pool(name="tmp", bufs=2))

    def load(name, pool=sb):
        shape = io[name].shape
        t = pool.tile(list(shape), io[name].dtype, tag=name, name=name + "_sb")
        nc.sync.dma_start(t[:, :], io[name][:, :])
        return t

    # scan-critical tensors first so the recurrence starts ASAP
    wc = []
    wh = []
    whp = []
    for k in range(2):
        wck = sb.tile([128, 256], BF16, tag=f"wc{k}")
        nc.sync.dma_start(wck[:, :], io["wc"][128 * k:128 * k + 128, :])
        wc.append(wck)
        whk = sb.tile([128, 6], BF16, tag=f"wh{k}")
        nc.sync.dma_start(whk[:, :], io["wh"][128 * k:128 * k + 128, :])
        wh.append(whk)
        whpk = sb.tile([128, 128], BF16, tag=f"whp{k}")
        nc.sync.dma_start(whpk[:, :], io["whp"][128 * k:128 * k + 128, :])
        whp.append(whpk)
    db = load("db")
    x_aug = load("x_aug")
    neg1 = load("neg1")
    # post-phase constants (not needed until the scan drains)
    x_f32 = load("x_f32")
    bias_head = load("bias_head")
    bias_head_p1 = load("bias_head_p1")
    bias_y = load("bias_y")
    bias_y_p1 = load("bias_y_p1")
    pattS = load("pattS")
    pattD = load("pattD")
    pattL0 = load("pattL0")
    pattW2 = load("pattW2")
    ones128 = load("ones128")

    # headsP SBUF store: 4 tiles [96, 512] f32, tile tau holds steps
    # 16*tau + tt at partitions 6*tt..6*tt+5
    headsP = [sb.tile([96, 512], F32, tag=f"headsP{i}", name=f"headsP{i}")
              for i in range(4)]

    nh_prev = []
    for h in range(2):
        nht = st.tile([128, 512], BF16, tag=f"nh{h}", name=f"nh{h}_init")
        nc.sync.dma_start(nht[:, :], io["nh0"][:, :])
        nh_prev.append(nht)

    def emit_head(t_h, hh, hps, kk):
        # head matmul k-chunk kk for half hh of step t_h
        s = t_h % 4
        if s == 0:
            # zero-padded weights write the full bank partition-wise:
            # initializes junk partitions so the eviction reads defined data
            dst = hps[:, 256 * hh:256 * hh + 256]
            w = whp[kk]
        else:
            dst = hps[32 * s:32 * s + 6, 256 * hh:256 * hh + 256]
            w = wh[kk]
        nc.tensor.matmul(dst, w[:, :], nh_prev[hh][:, 256 * kk:256 * kk + 256],
                         start=(kk == 0), stop=(kk == 1),
                         tile_position=(0, 32 * s))

    def flush_heads(t_last, hps):
        # steps t_last-3 .. t_last live at offsets 32*s.  DMA cannot read
        # PSUM, so evict the whole bank via ACT copy, then shuffle
        # partitions with SBUF->SBUF DMAs.
        stg = tmp.tile([128, 512], F32, tag="hstage", name=f"hstage{t_last}")
        nc.vector.tensor_copy(stg[:, :], hps[:, :])
        for s in range(4):
            t_h = t_last - 3 + s
            tau, tt = t_h // 16, t_h % 16
            nc.sync.dma_start(headsP[tau][6 * tt:6 * tt + 6, :],
                              stg[32 * s:32 * s + 6, :])

    with tc.tile_pool(name="z", bufs=2, space="PSUM") as zpool, \
         tc.tile_pool(name="hp", bufs=4, space="PSUM") as hpool:
        hps = hpool.tile([128, 512], F32, tag="hps")

        for t in range(1, L):
            for hh in range(2):
                zt = zpool.tile([128, 512], F32, tag=f"z{hh}",
                                name=f"z{hh}_{t}")
                zm = [zt[:, 0:256], zt[:, 256:512]]
                xr = x_aug[0:3, 512 * (t - 1) + 256 * hh:
                           512 * (t - 1) + 256 * hh + 256]
                nhp = nh_prev[hh]
                # sequential accumulation groups (chunk m0 then m1);
                # input+bias and prev-step head matmuls carry no nh(t) dep
                nc.tensor.matmul(zm[0], db[:, 0:128], xr,
                                 start=True, stop=False)
                emit_head(t - 1, hh, hps, 0)
                nc.tensor.matmul(zm[0], wc[0][:, 0:128], nhp[:, 0:256],
                                 start=False, stop=False)
                nc.tensor.matmul(zm[0], wc[1][:, 0:128], nhp[:, 256:512],
                                 start=False, stop=True)
                nc.tensor.matmul(zm[1], db[:, 128:256], xr,
                                 start=True, stop=False)
                emit_head(t - 1, hh, hps, 1)
                nc.tensor.matmul(zm[1], wc[0][:, 128:256], nhp[:, 0:256],
                                 start=False, stop=False)
                nc.tensor.matmul(zm[1], wc[1][:, 128:256], nhp[:, 256:512],
                                 start=False, stop=True)

                e = tmp.tile([128, 512], BF16, tag=f"e{hh}",
                             name=f"e{hh}_{t}")
                nc.scalar.activation(e[:, :], zt[:, :], AF.Exp,
                                     bias=neg1[:, 0:1])
                nh = st.tile([128, 512], BF16, tag=f"nh{hh}",
                             name=f"nh{hh}_{t}")
                # fused elu tail: nh = min(max(z+1, 1), exp(z))
                nc.vector.scalar_tensor_tensor(nh[:, :], zt[:, :], 1.0,
                                               e[:, :], OP.max, OP.min)
                nh_prev[hh] = nh

            if t % 4 == 0:
                # heads t-4..t-1 are complete in this bank now
                flush_heads(t - 1, hps)
                hps = hpool.tile([128, 512], F32, tag="hps",
                                 name=f"hps_{t}")

        for hh in range(2):
            for kk in range(2):
                emit_head(63, hh, hps, kk)
        flush_heads(63, hps)

    # ------------------------------------------------------------- post phase
    ppool = ctx.enter_context(tc.tile_pool(name="pp", bufs=1))
    pps = ctx.enter_context(tc.tile_pool(name="pps", bufs=1, space="PSUM"))

    nhd, E = [], []
    for tau in range(4):
        e = ppool.tile([96, 512], BF16, tag=f"he{tau}")
        nc.scalar.activation(e[:, :], headsP[tau][:, :], AF.Exp,
                             bias=bias_head[:, 0:1])
        u = ppool.tile([96, 512], BF16, tag=f"hu{tau}")
        nc.vector.tensor_scalar(u[:, :], headsP[tau][:, :],
                                bias_head_p1[:, 0:1], 1.0, OP.add, OP.max)
        nh_t = ppool.tile([96, 512], BF16, tag=f"nhd{tau}")
        nc.vector.tensor_tensor(nh_t[:, :], e[:, :], u[:, :], OP.min)
        nhd.append(nh_t)
        # clamped copy for exp (phase rows would overflow exp otherwise)
        vcl = ppool.tile([96, 512], BF16, tag=f"vcl{tau}")
        nc.vector.tensor_scalar(vcl[:, :], nh_t[:, :], 60.0, None, OP.min)
        ee = ppool.tile([96, 512], BF16, tag=f"E{tau}")
        nc.scalar.activation(ee[:, :], vcl[:, :], AF.Exp)
        E.append(ee)

    S = pps.tile([64, 512], F32, tag="S")
    D = pps.tile([64, 512], F32, tag="D")
    L0 = pps.tile([64, 512], F32, tag="L0")
    y = [pps.tile([128, 512], F32, tag=f"y{h}", name=f"y{h}") for h in range(2)]
    for tau in range(4):
        st_, sp = (tau == 0), (tau == 3)
        nc.tensor.matmul(S, pattS[:, 64 * tau:64 * tau + 64], E[tau][:, :],
                         start=st_, stop=sp)
        nc.tensor.matmul(D, pattD[:, 64 * tau:64 * tau + 64],
                         nhd[tau][:, :], start=st_, stop=sp)
        nc.tensor.matmul(L0, pattL0[:, 64 * tau:64 * tau + 64],
                         nhd[tau][:, :], start=st_, stop=sp)
        h = tau // 2
        nc.tensor.matmul(y[h], pattW2[:, 128 * tau:128 * tau + 128],
                         nhd[tau][:, :], start=(tau % 2 == 0),
                         stop=(tau % 2 == 1))

    lnS = ppool.tile([64, 512], F32, tag="lnS")
    nc.scalar.activation(lnS[:, :], S[:, :], AF.Ln)
    t1 = ppool.tile([64, 512], F32, tag="t1")
    nc.vector.tensor_tensor(t1[:, :], x_f32[:, :], D[:, :], OP.mult)
    t2 = ppool.tile([64, 512], F32, tag="t2")
    nc.vector.tensor_tensor(t2[:, :], t1[:, :], L0[:, :], OP.add)
    logp = ppool.tile([64, 512], BF16, tag="logp")
    nc.vector.tensor_tensor(logp[:, :], t2[:, :], lnS[:, :], OP.subtract)

    sum_r = pps.tile([1, 512], F32, tag="sum_r")
    nc.tensor.matmul(sum_r, ones128[0:64, :], logp[:, :],
                     start=True, stop=True)

    sum_i = pps.tile([1, 512], F32, tag="sum_i")
    for h in range(2):
        e = ppool.tile([128, 512], BF16, tag=f"ye{h}")
        nc.scalar.activation(e[:, :], y[h][:, :], AF.Exp, bias=bias_y[:, 0:1])
        u = ppool.tile([128, 512], BF16, tag=f"yu{h}")
        nc.vector.tensor_scalar(u[:, :], y[h][:, :], bias_y_p1[:, 0:1], 1.0,
                                OP.add, OP.max)
        nh2 = ppool.tile([128, 512], BF16, tag=f"nh2{h}")
        nc.vector.tensor_tensor(nh2[:, :], e[:, :], u[:, :], OP.min)
        nc.tensor.matmul(sum_i, ones128[:, :], nh2[:, :],
                         start=(h == 0), stop=(h == 1))

    out_r = ppool.tile([1, 512], F32, tag="out_r")
    nc.vector.tensor_copy(out_r[:, :], sum_r[:, :])
    out_i = ppool.tile([1, 512], F32, tag="out_i")
    nc.vector.tensor_copy(out_i[:, :], sum_i[:, :])
    nc.sync.dma_start(io["out"][0:1, :], out_r[:, :])
    nc.sync.dma_start(io["out"][1:2, :], out_i[:, :])


def build_program():
    nc = bacc.Bacc("TRN2", target_bir_lowering=False, debug=False,
                   num_devices=NCORES)
    io = {}
    for name, shape, dt in _IN_SPECS:
        io[name] = nc.dram_tensor(name, list(shape), dt,
                                  kind="ExternalInput").ap()
    io["out"] = nc.dram_tensor("out", [2, BL], F32,
                               kind="ExternalOutput").ap()
    with tile.TileContext(nc) as tc:
        with ExitStack() as ctx:
            _build_kernel(ctx, tc, io)
    nc.compile()
    return nc


def make_in_maps(x, W_in, W_carry, b_carry, W_prob, b_prob, W_ph1, b_ph1,
                 W_ph2, b_ph2):
    consts = _host_constants(W_in, W_carry, b_carry, W_prob, b_prob, W_ph1,
                             b_ph1, W_ph2, b_ph2)
    in_maps = []
    for c in range(NCORES):
        xs = np.ascontiguousarray(x[c * BL:(c + 1) * BL].T)  # [64, 512] i32
        m = dict(consts)
        xa = np.ones((3, L * BL), np.float32)
        xa[0] = xs.astype(np.float32).reshape(-1)
        m["x_aug"] = xa.astype(BF)
        m["x_f32"] = xs.astype(np.float32)
        in_maps.append(m)
    return in_maps


_PROGRAM = None


def kernel(x, W_in, W_carry, b_carry, W_prob, b_prob, W_ph1, b_ph1, W_ph2,
           b_ph2):
    global _PROGRAM
    x = np.asarray(x)
    in_maps = make_in_maps(x, np.asarray(W_in), np.asarray(W_carry),
                           np.asarray(b_carry), np.asarray(W_prob),
                           np.asarray(b_prob), np.asarray(W_ph1),
                           np.asarray(b_ph1), np.asarray(W_ph2),
                           np.asarray(b_ph2))
    if _PROGRAM is None:
        _PROGRAM = build_program()
    res = run_bass_kernel_spmd(_PROGRAM, in_maps, core_ids=list(range(NCORES)))
    outs = [np.asarray(res.results[c]["out"]) for c in range(NCORES)]
    real = 0.5 * np.concatenate([o[0] for o in outs])
    imag = (np.concatenate([o[1] for o in outs]) - 256.0) / 256.0
    return (real + 1j * imag).astype(np.complex64)
